# revision 18
# baseline (speedup 1.0000x reference)
"""Trainium2 Bass kernel for CustomMultiHeadAttention.

Problem: T=S=1024, B=8, C=1024, H=16 heads, head_dim=64, fp32.
  q = (query @ Wq.T + bq) * scale ; k = key @ Wk.T + bk ; v = value @ Wv.T + bv
  scores = q @ k.T per (b, h); softmax over s (with key_padding_mask);
  out = (attn @ v) @ Wo.T + bo

Sharding: batch-parallel — core b owns batch element b (8 cores, SPMD, no
collectives; projection weights replicated).

Per-core device algorithm (all matmuls in float32r — full PE rate):
  Phase A: projections.
    qT[o,t] (feature-major)  = WqT-tile.T @ xqT    (+bq per-partition)
    kT[o,t] likewise.
    v[s,o]  (token-major)    = xvT-tile.T @ WvT    (+bv via rank-1 matmul)
  Phase B: per head-pair j (heads 2j at partitions 0:64, 2j+1 at 64:128):
    scoresT[s,t] = kT_h-slice.T @ qT_h  (K=64, row-packed pairs)
    eT = Exp(SCALE*scoresT + maskbias[s])   (ACT; mask folded into bias)
    av = [v_h | ones].T @ eT  -> rows = unnormalized out^T, +1 row = Z[t]
    Z -> DRAM -> partition-broadcast back -> reciprocal -> attnT = num * (1/Z)
  Phase C: out[t,o] = attnT-tile.T @ WoT (+bo via rank-1), DMA from PSUM.
"""

import numpy as np

import concourse.bass as bass
import concourse.tile as tile
from concourse import bacc, mybir
from concourse.bass_utils import run_bass_kernel_spmd

F32 = mybir.dt.float32
F32R = mybir.dt.float32r

T = 1024
S = 1024
B = 8
C = 1024
H = 16
HD = 64
SCALE = float(HD) ** -0.5

N_CORES = 8




def _build(bq_any: bool, bk_any: bool, bv_any: bool, bo_any: bool):
    """Build the SPMD Bass program for one core's batch slice."""
    nc = bacc.Bacc(
        "TRN2",
        target_bir_lowering=False,
        debug=False,
        num_devices=N_CORES,
    )

    xq_d = nc.dram_tensor("xq_t", [C, T], F32R, kind="ExternalInput")
    xk_d = nc.dram_tensor("xk_t", [C, S], F32R, kind="ExternalInput")
    xv_d = nc.dram_tensor("xv_t", [C, S], F32R, kind="ExternalInput")
    wq_d = nc.dram_tensor("wq_t", [C, C], F32R, kind="ExternalInput")
    wk_d = nc.dram_tensor("wk_t", [C, C], F32R, kind="ExternalInput")
    wv_d = nc.dram_tensor("wv_t", [C, C], F32R, kind="ExternalInput")
    wo_d = nc.dram_tensor("wo_t", [C, C], F32R, kind="ExternalInput")
    bq_d = nc.dram_tensor("bq_c", [128, 8], F32, kind="ExternalInput")
    bk_d = nc.dram_tensor("bk_c", [128, 8], F32, kind="ExternalInput")
    bv_d = nc.dram_tensor("bv_r", [1, C], F32R, kind="ExternalInput")
    bo_d = nc.dram_tensor("bo_r", [1, C], F32R, kind="ExternalInput")
    mb_d = nc.dram_tensor("maskb", [128, 8], F32, kind="ExternalInput")
    on_d = nc.dram_tensor("ones_c", [128, 8, H, 1], F32R, kind="ExternalInput")
    out_d = nc.dram_tensor("out", [T, C], F32, kind="ExternalOutput")
    z_d = nc.dram_tensor("zscratch", [H, T], F32, kind="Internal")
    z2_d = nc.dram_tensor("zscratch2", [H, T], F32, kind="Internal")

    Exp = mybir.ActivationFunctionType.Exp

    with tile.TileContext(nc) as tc:
        with (
            tc.tile_pool(name="singles", bufs=1) as singles,
            tc.tile_pool(name="wpool", bufs=10) as wpool,
            tc.tile_pool(name="acts", bufs=1) as acts,
            tc.tile_pool(name="stream", bufs=3) as stream,
        ):
            # --- small constants ---
            maskb = singles.tile([128, 8], F32)
            nc.gpsimd.dma_start(maskb, mb_d.ap())
            bq_sb = singles.tile([128, 8], F32)
            nc.gpsimd.dma_start(bq_sb, bq_d.ap())
            bk_sb = singles.tile([128, 8], F32)
            nc.gpsimd.dma_start(bk_sb, bk_d.ap())
            if bv_any or bo_any:
                ones1 = singles.tile([1, 128], F32R)
                nc.sync.dma_start(ones1, on_d.ap().rearrange("p a b c -> p (a b c)")[0:1, 0:128])
            if bv_any:
                bv_sb = singles.tile([1, C], F32R)
                nc.sync.dma_start(bv_sb, bv_d.ap())
            if bo_any:
                bo_sb = singles.tile([1, C], F32R)
                nc.sync.dma_start(bo_sb, bo_d.ap())

            # --- persistent activations ---
            # qT_j / later attnT_j share the "qa" slots ([128, 1024] each).
            qT = [
                acts.tile([128, T], F32R, tag="qa", bufs=8, name=f"qT{j}")
                for j in range(8)
            ]
            kT = [
                acts.tile([128, S], F32R, tag="kt", bufs=8, name=f"kT{j}")
                for j in range(8)
            ]
            # v token-major, 65-wide head slots: cols 0..63 = v dims, col 64 = ones
            # (the ones column makes the PV matmul also emit Z = sum_s e as row 64).
            v_sb = acts.tile([128, 8, H, 65], F32R, tag="v", bufs=1)
            ones_col = singles.tile([128, 1], F32R)
            nc.gpsimd.dma_start(
                ones_col, on_d.ap().rearrange("p a b c -> p (a b c)")[:, 0:1]
            )
            nc.vector.tensor_copy(
                v_sb[:, :, :, 64:65], ones_col[:, :, None, None].to_broadcast((128, 8, H, 1))
            )

            # ---------------- Phase A: projections ----------------
            with tc.tile_pool(name="psA", bufs=8, space="PSUM") as psA:

                def proj_featmajor(x_d, w_d, b_sb, outs, wname):
                    # outs[j][o_p, t] = sum_i W.T[i, o] x^T[i, t]  (+ b[o])
                    w_sb = []
                    for k in range(8):
                        wt = wpool.tile([128, C], F32R, tag="w", name=f"{wname}{k}")
                        nc.gpsimd.dma_start(wt, w_d.ap()[k * 128 : (k + 1) * 128, :])
                        w_sb.append(wt)
                    for tci in range(2):
                        ps = [
                            psA.tile([128, 512], F32, tag="pa", name=f"ps{wname}{tci}_{j}")
                            for j in range(8)
                        ]
                        for k in range(8):
                            xch = stream.tile([128, 512], F32R, tag="xch", bufs=4)
                            nc.sync.dma_start(
                                xch,
                                x_d.ap()[
                                    k * 128 : (k + 1) * 128,
                                    tci * 512 : (tci + 1) * 512,
                                ],
                            )
                            for j in range(8):
                                nc.tensor.matmul(
                                    ps[j],
                                    (w_sb[k][:, j * 128 : (j + 1) * 128]),
                                    (xch),
                                    start=(k == 0),
                                    stop=(k == 7),
                                )
                        for j in range(8):
                            nc.vector.tensor_scalar_add(
                                outs[j][:, tci * 512 : (tci + 1) * 512],
                                ps[j],
                                b_sb[:, j : j + 1],
                            )

                proj_featmajor(xq_d, wq_d, bq_sb, qT, "wq")
                proj_featmajor(xk_d, wk_d, bk_sb, kT, "wk")

                # v token-major: v[s, o] = sum_i x^T[i, s] W.T[i, o] (+ bv[o])
                wv_sb = []
                for k in range(8):
                    wt = wpool.tile([128, C], F32R, tag="w", name=f"wv{k}")
                    nc.gpsimd.dma_start(wt, wv_d.ap()[k * 128 : (k + 1) * 128, :])
                    wv_sb.append(wt)
                for sh in range(2):
                    xvh = []
                    for k in range(8):
                        xt = stream.tile(
                            [128, 512], F32R, tag="xvh", bufs=10, name=f"xvh{sh}_{k}"
                        )
                        nc.sync.dma_start(
                            xt,
                            xv_d.ap()[
                                k * 128 : (k + 1) * 128, sh * 512 : (sh + 1) * 512
                            ],
                        )
                        xvh.append(xt)
                    for si in range(4):
                        s = sh * 4 + si
                        for oc in range(2):
                            psv = psA.tile(
                                [128, 512], F32, tag="pa", name=f"psv{s}_{oc}"
                            )
                            for k in range(8):
                                nc.tensor.matmul(
                                    psv,
                                    (xvh[k][:, si * 128 : (si + 1) * 128]),
                                    (wv_sb[k][:, oc * 512 : (oc + 1) * 512]),
                                    start=(k == 0),
                                    stop=(k == 7 and not bv_any),
                                )
                            if bv_any:
                                nc.tensor.matmul(
                                    psv,
                                    (ones1[0:1, 0:128]),
                                    (bv_sb[0:1, oc * 512 : (oc + 1) * 512]),
                                    start=False,
                                    stop=True,
                                )
                            # scatter into per-head 65-wide slots (cols 0..63)
                            nc.vector.tensor_copy(
                                v_sb[:, s, 8 * oc : 8 * oc + 8, 0:64],
                                psv.rearrange("p (h d) -> p h d", d=64),
                            )

            # ---------------- Phase B: attention per head-pair ----------------
            attnT = []
            with tc.tile_pool(name="psB", bufs=2, space="PSUM") as psB:
                for j in range(8):
                    h0, h1 = 2 * j, 2 * j + 1
                    avA = psB.tile([65, T], F32, tag="av", name=f"avA{j}")
                    avB = psB.tile([65, T], F32, tag="av", name=f"avB{j}")
                    for s in range(8):
                        scA = psB.tile([128, T], F32, tag="sc", name=f"scA{j}_{s}")
                        scB = psB.tile([128, T], F32, tag="sc", name=f"scB{j}_{s}")
                        for tcn in range(2):
                            tsl = slice(tcn * 512, (tcn + 1) * 512)
                            nc.tensor.matmul(
                                scA[:, tsl],
                                (kT[j][0:64, s * 128 : (s + 1) * 128]),
                                (qT[j][0:64, tsl]),
                                start=True,
                                stop=True,
                            )
                            nc.tensor.matmul(
                                scB[:, tsl],
                                (kT[j][64:128, s * 128 : (s + 1) * 128]),
                                (qT[j][64:128, tsl]),
                                start=True,
                                stop=True,
                            )
                        eA = stream.tile([128, T], F32R, tag="e", bufs=3, name=f"eA{j}_{s}")
                        eB = stream.tile([128, T], F32R, tag="e", bufs=3, name=f"eB{j}_{s}")
                        nc.scalar.activation(
                            eA, scA, Exp, bias=maskb[:, s : s + 1], scale=SCALE
                        )
                        nc.scalar.activation(
                            eB, scB, Exp, bias=maskb[:, s : s + 1], scale=SCALE
                        )
                        for tcn in range(2):
                            tsl = slice(tcn * 512, (tcn + 1) * 512)
                            nc.tensor.matmul(
                                avA[:, tsl],
                                (v_sb[:, s, h0, :]),
                                (eA[:, tsl]),
                                start=(s == 0),
                                stop=(s == 7),
                            )
                            nc.tensor.matmul(
                                avB[:, tsl],
                                (v_sb[:, s, h1, :]),
                                (eB[:, tsl]),
                                start=(s == 0),
                                stop=(s == 7),
                            )
                    # stash Z rows to DRAM; copy unnormalized num into attnT
                    zrowA = stream.tile([1, T], F32, tag="zz", bufs=3, name=f"zrowA{j}")
                    zrowB = stream.tile([1, T], F32, tag="zz", bufs=3, name=f"zrowB{j}")
                    nc.vector.tensor_copy(zrowA, avA[64:65, :])
                    nc.vector.tensor_copy(zrowB, avB[64:65, :])
                    nc.sync.dma_start(z_d.ap()[h0 : h0 + 1, :], zrowA)
                    nc.sync.dma_start(z_d.ap()[h1 : h1 + 1, :], zrowB)
                    at = acts.tile([128, T], F32R, tag="qa", bufs=8, name=f"attnT{j}")
                    nc.vector.tensor_copy(at[0:64, :], avA[0:64, :])
                    # odd head: copy at base 0, then DMA partition-shift to 64:128
                    tmpB = stream.tile([64, T], F32R, tag="e", bufs=3, name=f"tmpB{j}")
                    nc.vector.tensor_copy(tmpB, avB[0:64, :])
                    nc.sync.dma_start(at[64:128, :], tmpB)
                    attnT.append(at)
                    if j in (3, 7):
                        # batched normalize for pairs lo..j: one fast reciprocal
                        # for 8 heads, then broadcast 1/Z and scale attnT
                        lo = j - 3
                        zall = stream.tile(
                            [8, T], F32, tag="zz", bufs=3, name=f"zall{lo}"
                        )
                        nc.sync.dma_start(zall, z_d.ap()[2 * lo : 2 * lo + 8, :])
                        nc.vector.reciprocal_approx_fast(out=zall, in_=zall)
                        nc.sync.dma_start(z2_d.ap()[2 * lo : 2 * lo + 8, :], zall)
                        for jj in range(lo, j + 1):
                            zbc = stream.tile(
                                [128, T], F32, tag="zz", bufs=3, name=f"zbcn{jj}"
                            )
                            nc.sync.dma_start(
                                zbc[0:64, :],
                                z2_d.ap()[2 * jj : 2 * jj + 1, :].to_broadcast((64, T)),
                            )
                            nc.sync.dma_start(
                                zbc[64:128, :],
                                z2_d.ap()[2 * jj + 1 : 2 * jj + 2, :].to_broadcast(
                                    (64, T)
                                ),
                            )
                            nc.vector.tensor_mul(attnT[jj], attnT[jj], zbc)

            # ---------------- Phase C: output projection ----------------
            wo_sb = []
            for k in range(8):
                wt = wpool.tile([128, C], F32R, tag="w", name=f"wo{k}")
                nc.gpsimd.dma_start(wt, wo_d.ap()[k * 128 : (k + 1) * 128, :])
                wo_sb.append(wt)
            with tc.tile_pool(name="psC", bufs=4, space="PSUM") as psC:
                for tt in range(8):
                    for oc in range(2):
                        pso = psC.tile([128, 512], F32, tag="pc", name=f"pso{tt}_{oc}")
                        for it in range(8):
                            nc.tensor.matmul(
                                pso,
                                (attnT[it][:, tt * 128 : (tt + 1) * 128]),
                                (wo_sb[it][:, oc * 512 : (oc + 1) * 512]),
                                start=(it == 0),
                                stop=(it == 7 and not bo_any),
                            )
                        if bo_any:
                            nc.tensor.matmul(
                                pso,
                                (ones1[0:1, 0:128]),
                                (bo_sb[0:1, oc * 512 : (oc + 1) * 512]),
                                start=False,
                                stop=True,
                            )
                        osb = stream.tile(
                            [128, 512], F32, tag="xch", bufs=4, name=f"osb{tt}_{oc}"
                        )
                        nc.vector.tensor_copy(osb, pso)
                        nc.sync.dma_start(
                            out_d.ap()[
                                tt * 128 : (tt + 1) * 128, oc * 512 : (oc + 1) * 512
                            ],
                            osb,
                        )

    nc.compile()
    return nc


_last_results = None


def kernel(
    query,
    key,
    value,
    key_padding_mask,
    Wq,
    bq,
    Wk,
    bk,
    Wv,
    bv,
    Wo,
    bo,
    _trace=False,
):
    global _last_results
    query = np.asarray(query, np.float32)
    key = np.asarray(key, np.float32)
    value = np.asarray(value, np.float32)
    mask = np.asarray(key_padding_mask, bool)
    Wq = np.asarray(Wq, np.float32)
    Wk = np.asarray(Wk, np.float32)
    Wv = np.asarray(Wv, np.float32)
    Wo = np.asarray(Wo, np.float32)
    bq = np.asarray(bq, np.float32)
    bk = np.asarray(bk, np.float32)
    bv = np.asarray(bv, np.float32)
    bo = np.asarray(bo, np.float32)

    nc = _build(
        bq_any=bool(bq.any()),
        bk_any=bool(bk.any()),
        bv_any=bool(bv.any()),
        bo_any=bool(bo.any()),
    )

    # weight pre-layout (shared across cores): W.T, contiguous [c_in, c_out]
    wqT = np.ascontiguousarray(Wq.T)
    wkT = np.ascontiguousarray(Wk.T)
    wvT = np.ascontiguousarray(Wv.T)
    woT = np.ascontiguousarray(Wo.T)
    bq_c = np.ascontiguousarray(bq.reshape(8, 128).T)
    bk_c = np.ascontiguousarray(bk.reshape(8, 128).T)
    bv_r = bv.reshape(1, C)
    bo_r = bo.reshape(1, C)

    in_maps = []
    for b in range(N_CORES):
        maskbias = np.where(mask[b], np.float32(-1e30), np.float32(0.0)).astype(
            np.float32
        )
        in_maps.append(
            {
                "xq_t": np.ascontiguousarray(query[:, b, :].T),
                "xk_t": np.ascontiguousarray(key[:, b, :].T),
                "xv_t": np.ascontiguousarray(value[:, b, :].T),
                "wq_t": wqT,
                "wk_t": wkT,
                "wv_t": wvT,
                "wo_t": woT,
                "bq_c": bq_c,
                "bk_c": bk_c,
                "bv_r": bv_r,
                "bo_r": bo_r,
                "maskb": np.ascontiguousarray(maskbias.reshape(8, 128).T),
                "ones_c": np.ones((128, 8, H, 1), np.float32),
            }
        )

    res = run_bass_kernel_spmd(
        nc,
        in_maps,
        core_ids=list(range(N_CORES)),
        trace=_trace,
    )
    _last_results = res
    out = np.stack([res.results[b]["out"] for b in range(N_CORES)], axis=1)
    return out.astype(np.float32)


# revision 19
# speedup vs baseline: 1.0142x; 1.0142x over previous
"""Trainium2 Bass kernel for CustomMultiHeadAttention.

Problem: T=S=1024, B=8, C=1024, H=16 heads, head_dim=64, fp32.
  q = (query @ Wq.T + bq) * scale ; k = key @ Wk.T + bk ; v = value @ Wv.T + bv
  scores = q @ k.T per (b, h); softmax over s (with key_padding_mask);
  out = (attn @ v) @ Wo.T + bo

Sharding: batch-parallel — core b owns batch element b (8 cores, SPMD, no
collectives; projection weights replicated).

Per-core device algorithm (all matmuls in float32r — full PE rate):
  Phase A: projections.
    qT[o,t] (feature-major)  = WqT-tile.T @ xqT    (+bq per-partition)
    kT[o,t] likewise.
    v[s,o]  (token-major)    = xvT-tile.T @ WvT    (+bv via rank-1 matmul)
  Phase B: per head-pair j (heads 2j at partitions 0:64, 2j+1 at 64:128):
    scoresT[s,t] = kT_h-slice.T @ qT_h  (K=64, row-packed pairs)
    eT = Exp(SCALE*scoresT + maskbias[s])   (ACT; mask folded into bias)
    av = [v_h | ones].T @ eT  -> rows = unnormalized out^T, +1 row = Z[t]
    Z -> DRAM -> partition-broadcast back -> reciprocal -> attnT = num * (1/Z)
  Phase C: out[t,o] = attnT-tile.T @ WoT (+bo via rank-1), DMA from PSUM.
"""

import numpy as np

import concourse.bass as bass
import concourse.tile as tile
from concourse import bacc, mybir
from concourse.bass_utils import run_bass_kernel_spmd

F32 = mybir.dt.float32
F32R = mybir.dt.float32r

T = 1024
S = 1024
B = 8
C = 1024
H = 16
HD = 64
SCALE = float(HD) ** -0.5

N_CORES = 8




def _build(bq_any: bool, bk_any: bool, bv_any: bool, bo_any: bool):
    """Build the SPMD Bass program for one core's batch slice."""
    nc = bacc.Bacc(
        "TRN2",
        target_bir_lowering=False,
        debug=False,
        num_devices=N_CORES,
    )

    xq_d = nc.dram_tensor("xq_t", [C, T], F32R, kind="ExternalInput")
    xk_d = nc.dram_tensor("xk_t", [C, S], F32R, kind="ExternalInput")
    xv_d = nc.dram_tensor("xv_t", [C, S], F32R, kind="ExternalInput")
    wq_d = nc.dram_tensor("wq_t", [C, C], F32R, kind="ExternalInput")
    wk_d = nc.dram_tensor("wk_t", [C, C], F32R, kind="ExternalInput")
    wv_d = nc.dram_tensor("wv_t", [C, C], F32R, kind="ExternalInput")
    wo_d = nc.dram_tensor("wo_t", [C, C], F32R, kind="ExternalInput")
    bq_d = nc.dram_tensor("bq_c", [128, 8], F32, kind="ExternalInput")
    bk_d = nc.dram_tensor("bk_c", [128, 8], F32, kind="ExternalInput")
    bv_d = nc.dram_tensor("bv_r", [1, C], F32R, kind="ExternalInput")
    bo_d = nc.dram_tensor("bo_r", [1, C], F32R, kind="ExternalInput")
    mb_d = nc.dram_tensor("maskb", [128, 8], F32, kind="ExternalInput")
    on_d = nc.dram_tensor("ones_c", [128, 8, H, 1], F32R, kind="ExternalInput")
    out_d = nc.dram_tensor("out", [T, C], F32, kind="ExternalOutput")
    z_d = nc.dram_tensor("zscratch", [H, T], F32, kind="Internal")
    z2_d = nc.dram_tensor("zscratch2", [H, T], F32, kind="Internal")

    Exp = mybir.ActivationFunctionType.Exp

    with tile.TileContext(nc) as tc:
        with (
            tc.tile_pool(name="singles", bufs=1) as singles,
            tc.tile_pool(name="wpool", bufs=10) as wpool,
            tc.tile_pool(name="acts", bufs=1) as acts,
            tc.tile_pool(name="stream", bufs=3) as stream,
        ):
            # --- small constants ---
            maskb = singles.tile([128, 8], F32)
            nc.gpsimd.dma_start(maskb, mb_d.ap())
            bq_sb = singles.tile([128, 8], F32)
            nc.gpsimd.dma_start(bq_sb, bq_d.ap())
            bk_sb = singles.tile([128, 8], F32)
            nc.gpsimd.dma_start(bk_sb, bk_d.ap())
            if bv_any or bo_any:
                ones1 = singles.tile([1, 128], F32R)
                nc.sync.dma_start(ones1, on_d.ap().rearrange("p a b c -> p (a b c)")[0:1, 0:128])
            if bv_any:
                bv_sb = singles.tile([1, C], F32R)
                nc.sync.dma_start(bv_sb, bv_d.ap())
            if bo_any:
                bo_sb = singles.tile([1, C], F32R)
                nc.sync.dma_start(bo_sb, bo_d.ap())

            # --- persistent activations ---
            # qT_j / later attnT_j share the "qa" slots ([128, 1024] each).
            qT = [
                acts.tile([128, T], F32R, tag="qa", bufs=8, name=f"qT{j}")
                for j in range(8)
            ]
            kT = [
                acts.tile([128, S], F32R, tag="kt", bufs=8, name=f"kT{j}")
                for j in range(8)
            ]
            # v token-major, 65-wide head slots: cols 0..63 = v dims, col 64 = ones
            # (the ones column makes the PV matmul also emit Z = sum_s e as row 64).
            v_sb = acts.tile([128, 8, H, 65], F32R, tag="v", bufs=1)
            ones_col = singles.tile([128, 1], F32R)
            nc.gpsimd.dma_start(
                ones_col, on_d.ap().rearrange("p a b c -> p (a b c)")[:, 0:1]
            )
            nc.vector.tensor_copy(
                v_sb[:, :, :, 64:65], ones_col[:, :, None, None].to_broadcast((128, 8, H, 1))
            )

            # ---------------- Phase A: projections ----------------
            with tc.tile_pool(name="psA", bufs=8, space="PSUM") as psA:

                def proj_featmajor(x_d, w_d, b_sb, outs, wname):
                    # outs[j][o_p, t] = sum_i W.T[i, o] x^T[i, t]  (+ b[o])
                    w_sb = []
                    for k in range(8):
                        wt = wpool.tile([128, C], F32R, tag="w", name=f"{wname}{k}")
                        nc.sync.dma_start(wt, w_d.ap()[k * 128 : (k + 1) * 128, :])
                        w_sb.append(wt)
                    for tci in range(2):
                        ps = [
                            psA.tile([128, 512], F32, tag="pa", name=f"ps{wname}{tci}_{j}")
                            for j in range(8)
                        ]
                        for k in range(8):
                            xch = stream.tile([128, 512], F32R, tag="xch", bufs=4)
                            nc.sync.dma_start(
                                xch,
                                x_d.ap()[
                                    k * 128 : (k + 1) * 128,
                                    tci * 512 : (tci + 1) * 512,
                                ],
                            )
                            for j in range(8):
                                nc.tensor.matmul(
                                    ps[j],
                                    (w_sb[k][:, j * 128 : (j + 1) * 128]),
                                    (xch),
                                    start=(k == 0),
                                    stop=(k == 7),
                                )
                        for j in range(8):
                            nc.vector.tensor_scalar_add(
                                outs[j][:, tci * 512 : (tci + 1) * 512],
                                ps[j],
                                b_sb[:, j : j + 1],
                            )

                proj_featmajor(xq_d, wq_d, bq_sb, qT, "wq")
                proj_featmajor(xk_d, wk_d, bk_sb, kT, "wk")

                # v token-major: v[s, o] = sum_i x^T[i, s] W.T[i, o] (+ bv[o])
                wv_sb = []
                for k in range(8):
                    wt = wpool.tile([128, C], F32R, tag="w", name=f"wv{k}")
                    nc.sync.dma_start(wt, wv_d.ap()[k * 128 : (k + 1) * 128, :])
                    wv_sb.append(wt)
                for sh in range(2):
                    xvh = []
                    for k in range(8):
                        xt = stream.tile(
                            [128, 512], F32R, tag="xvh", bufs=10, name=f"xvh{sh}_{k}"
                        )
                        nc.sync.dma_start(
                            xt,
                            xv_d.ap()[
                                k * 128 : (k + 1) * 128, sh * 512 : (sh + 1) * 512
                            ],
                        )
                        xvh.append(xt)
                    for si in range(4):
                        s = sh * 4 + si
                        for oc in range(2):
                            psv = psA.tile(
                                [128, 512], F32, tag="pa", name=f"psv{s}_{oc}"
                            )
                            for k in range(8):
                                nc.tensor.matmul(
                                    psv,
                                    (xvh[k][:, si * 128 : (si + 1) * 128]),
                                    (wv_sb[k][:, oc * 512 : (oc + 1) * 512]),
                                    start=(k == 0),
                                    stop=(k == 7 and not bv_any),
                                )
                            if bv_any:
                                nc.tensor.matmul(
                                    psv,
                                    (ones1[0:1, 0:128]),
                                    (bv_sb[0:1, oc * 512 : (oc + 1) * 512]),
                                    start=False,
                                    stop=True,
                                )
                            # scatter into per-head 65-wide slots (cols 0..63)
                            nc.vector.tensor_copy(
                                v_sb[:, s, 8 * oc : 8 * oc + 8, 0:64],
                                psv.rearrange("p (h d) -> p h d", d=64),
                            )

            # ---------------- Phase B: attention per head-pair ----------------
            attnT = []
            with tc.tile_pool(name="psB", bufs=2, space="PSUM") as psB:
                for j in range(8):
                    h0, h1 = 2 * j, 2 * j + 1
                    avA = psB.tile([65, T], F32, tag="av", name=f"avA{j}")
                    avB = psB.tile([65, T], F32, tag="av", name=f"avB{j}")
                    for s in range(8):
                        scA = psB.tile([128, T], F32, tag="sc", name=f"scA{j}_{s}")
                        scB = psB.tile([128, T], F32, tag="sc", name=f"scB{j}_{s}")
                        for tcn in range(2):
                            tsl = slice(tcn * 512, (tcn + 1) * 512)
                            nc.tensor.matmul(
                                scA[:, tsl],
                                (kT[j][0:64, s * 128 : (s + 1) * 128]),
                                (qT[j][0:64, tsl]),
                                start=True,
                                stop=True,
                            )
                            nc.tensor.matmul(
                                scB[:, tsl],
                                (kT[j][64:128, s * 128 : (s + 1) * 128]),
                                (qT[j][64:128, tsl]),
                                start=True,
                                stop=True,
                            )
                        eA = stream.tile([128, T], F32R, tag="e", bufs=4, name=f"eA{j}_{s}")
                        eB = stream.tile([128, T], F32R, tag="e", bufs=4, name=f"eB{j}_{s}")
                        nc.scalar.activation(
                            eA, scA, Exp, bias=maskb[:, s : s + 1], scale=SCALE
                        )
                        nc.scalar.activation(
                            eB, scB, Exp, bias=maskb[:, s : s + 1], scale=SCALE
                        )
                        for tcn in range(2):
                            tsl = slice(tcn * 512, (tcn + 1) * 512)
                            nc.tensor.matmul(
                                avA[:, tsl],
                                (v_sb[:, s, h0, :]),
                                (eA[:, tsl]),
                                start=(s == 0),
                                stop=(s == 7),
                            )
                            nc.tensor.matmul(
                                avB[:, tsl],
                                (v_sb[:, s, h1, :]),
                                (eB[:, tsl]),
                                start=(s == 0),
                                stop=(s == 7),
                            )
                    # stash Z rows to DRAM; copy unnormalized num into attnT
                    zrowA = stream.tile([1, T], F32, tag="zz", bufs=3, name=f"zrowA{j}")
                    zrowB = stream.tile([1, T], F32, tag="zz", bufs=3, name=f"zrowB{j}")
                    nc.vector.tensor_copy(zrowA, avA[64:65, :])
                    nc.vector.tensor_copy(zrowB, avB[64:65, :])
                    nc.sync.dma_start(z_d.ap()[h0 : h0 + 1, :], zrowA)
                    nc.sync.dma_start(z_d.ap()[h1 : h1 + 1, :], zrowB)
                    at = acts.tile([128, T], F32R, tag="qa", bufs=8, name=f"attnT{j}")
                    nc.vector.tensor_copy(at[0:64, :], avA[0:64, :])
                    # odd head: copy at base 0, then DMA partition-shift to 64:128
                    tmpB = stream.tile([64, T], F32R, tag="tmpb", bufs=1, name=f"tmpB{j}")
                    nc.vector.tensor_copy(tmpB, avB[0:64, :])
                    nc.sync.dma_start(at[64:128, :], tmpB)
                    attnT.append(at)
                    if j in (3, 7):
                        # batched normalize for pairs lo..j: one fast reciprocal
                        # for 8 heads, then broadcast 1/Z and scale attnT
                        lo = j - 3
                        zall = stream.tile(
                            [8, T], F32, tag="zz", bufs=3, name=f"zall{lo}"
                        )
                        nc.sync.dma_start(zall, z_d.ap()[2 * lo : 2 * lo + 8, :])
                        nc.vector.reciprocal_approx_fast(out=zall, in_=zall)
                        nc.sync.dma_start(z2_d.ap()[2 * lo : 2 * lo + 8, :], zall)
                        for jj in range(lo, j + 1):
                            zbc = stream.tile(
                                [128, T], F32, tag="zz", bufs=3, name=f"zbcn{jj}"
                            )
                            nc.sync.dma_start(
                                zbc[0:64, :],
                                z2_d.ap()[2 * jj : 2 * jj + 1, :].to_broadcast((64, T)),
                            )
                            nc.sync.dma_start(
                                zbc[64:128, :],
                                z2_d.ap()[2 * jj + 1 : 2 * jj + 2, :].to_broadcast(
                                    (64, T)
                                ),
                            )
                            nc.vector.tensor_mul(attnT[jj], attnT[jj], zbc)

            # ---------------- Phase C: output projection ----------------
            wo_sb = []
            for k in range(8):
                wt = wpool.tile([128, C], F32R, tag="w", name=f"wo{k}")
                nc.sync.dma_start(wt, wo_d.ap()[k * 128 : (k + 1) * 128, :])
                wo_sb.append(wt)
            with tc.tile_pool(name="psC", bufs=4, space="PSUM") as psC:
                for tt in range(8):
                    for oc in range(2):
                        pso = psC.tile([128, 512], F32, tag="pc", name=f"pso{tt}_{oc}")
                        for it in range(8):
                            nc.tensor.matmul(
                                pso,
                                (attnT[it][:, tt * 128 : (tt + 1) * 128]),
                                (wo_sb[it][:, oc * 512 : (oc + 1) * 512]),
                                start=(it == 0),
                                stop=(it == 7 and not bo_any),
                            )
                        if bo_any:
                            nc.tensor.matmul(
                                pso,
                                (ones1[0:1, 0:128]),
                                (bo_sb[0:1, oc * 512 : (oc + 1) * 512]),
                                start=False,
                                stop=True,
                            )
                        osb = stream.tile(
                            [128, 512], F32, tag="xch", bufs=4, name=f"osb{tt}_{oc}"
                        )
                        nc.vector.tensor_copy(osb, pso)
                        nc.sync.dma_start(
                            out_d.ap()[
                                tt * 128 : (tt + 1) * 128, oc * 512 : (oc + 1) * 512
                            ],
                            osb,
                        )

    nc.compile()
    return nc


_last_results = None


def kernel(
    query,
    key,
    value,
    key_padding_mask,
    Wq,
    bq,
    Wk,
    bk,
    Wv,
    bv,
    Wo,
    bo,
    _trace=False,
):
    global _last_results
    query = np.asarray(query, np.float32)
    key = np.asarray(key, np.float32)
    value = np.asarray(value, np.float32)
    mask = np.asarray(key_padding_mask, bool)
    Wq = np.asarray(Wq, np.float32)
    Wk = np.asarray(Wk, np.float32)
    Wv = np.asarray(Wv, np.float32)
    Wo = np.asarray(Wo, np.float32)
    bq = np.asarray(bq, np.float32)
    bk = np.asarray(bk, np.float32)
    bv = np.asarray(bv, np.float32)
    bo = np.asarray(bo, np.float32)

    nc = _build(
        bq_any=bool(bq.any()),
        bk_any=bool(bk.any()),
        bv_any=bool(bv.any()),
        bo_any=bool(bo.any()),
    )

    # weight pre-layout (shared across cores): W.T, contiguous [c_in, c_out]
    wqT = np.ascontiguousarray(Wq.T)
    wkT = np.ascontiguousarray(Wk.T)
    wvT = np.ascontiguousarray(Wv.T)
    woT = np.ascontiguousarray(Wo.T)
    bq_c = np.ascontiguousarray(bq.reshape(8, 128).T)
    bk_c = np.ascontiguousarray(bk.reshape(8, 128).T)
    bv_r = bv.reshape(1, C)
    bo_r = bo.reshape(1, C)

    in_maps = []
    for b in range(N_CORES):
        maskbias = np.where(mask[b], np.float32(-1e30), np.float32(0.0)).astype(
            np.float32
        )
        in_maps.append(
            {
                "xq_t": np.ascontiguousarray(query[:, b, :].T),
                "xk_t": np.ascontiguousarray(key[:, b, :].T),
                "xv_t": np.ascontiguousarray(value[:, b, :].T),
                "wq_t": wqT,
                "wk_t": wkT,
                "wv_t": wvT,
                "wo_t": woT,
                "bq_c": bq_c,
                "bk_c": bk_c,
                "bv_r": bv_r,
                "bo_r": bo_r,
                "maskb": np.ascontiguousarray(maskbias.reshape(8, 128).T),
                "ones_c": np.ones((128, 8, H, 1), np.float32),
            }
        )

    res = run_bass_kernel_spmd(
        nc,
        in_maps,
        core_ids=list(range(N_CORES)),
        trace=_trace,
    )
    _last_results = res
    out = np.stack([res.results[b]["out"] for b in range(N_CORES)], axis=1)
    return out.astype(np.float32)


# revision 20
# speedup vs baseline: 1.3960x; 1.3764x over previous
"""Trainium2 Bass kernel for CustomMultiHeadAttention.

Problem: T=S=1024, B=8, C=1024, H=16 heads, head_dim=64, fp32.
  q = (query @ Wq.T + bq) * scale ; k = key @ Wk.T + bk ; v = value @ Wv.T + bv
  scores = q @ k.T per (b, h); softmax over s (with key_padding_mask);
  out = (attn @ v) @ Wo.T + bo

Sharding: batch-parallel — core b owns batch element b (8 cores, SPMD, no
collectives; projection weights replicated).

Per-core device algorithm (all matmuls in float32r — full PE rate):
  Phase A: projections.
    qT[o,t] (feature-major)  = WqT-tile.T @ xqT    (+bq per-partition)
    kT[o,t] likewise.
    v[s,o]  (token-major)    = xvT-tile.T @ WvT    (+bv via rank-1 matmul)
  Phase B: per head-pair j (heads 2j at partitions 0:64, 2j+1 at 64:128):
    scoresT[s,t] = kT_h-slice.T @ qT_h  (K=64, row-packed pairs)
    eT = Exp(SCALE*scoresT + maskbias[s])   (ACT; mask folded into bias)
    av = [v_h | ones].T @ eT  -> rows = unnormalized out^T, +1 row = Z[t]
    Z -> DRAM -> partition-broadcast back -> reciprocal -> attnT = num * (1/Z)
  Phase C: out[t,o] = attnT-tile.T @ WoT (+bo via rank-1), DMA from PSUM.
"""

import numpy as np

import concourse.bass as bass
import concourse.tile as tile
from concourse import bacc, mybir
from concourse.bass_utils import run_bass_kernel_spmd

F32 = mybir.dt.float32
F32R = mybir.dt.float32r
BF16 = mybir.dt.bfloat16

T = 1024
S = 1024
B = 8
C = 1024
H = 16
HD = 64
SCALE = float(HD) ** -0.5

N_CORES = 8




def _build(bq_any: bool, bk_any: bool, bv_any: bool, bo_any: bool):
    """Build the SPMD Bass program for one core's batch slice."""
    nc = bacc.Bacc(
        "TRN2",
        target_bir_lowering=False,
        debug=False,
        num_devices=N_CORES,
    )

    xq_d = nc.dram_tensor("xq_t", [C, T], BF16, kind="ExternalInput")
    xk_d = nc.dram_tensor("xk_t", [C, S], BF16, kind="ExternalInput")
    xv_d = nc.dram_tensor("xv_t", [C, S], BF16, kind="ExternalInput")
    wq_d = nc.dram_tensor("wq_t", [C, C], BF16, kind="ExternalInput")
    wk_d = nc.dram_tensor("wk_t", [C, C], BF16, kind="ExternalInput")
    wv_d = nc.dram_tensor("wv_t", [C, C], BF16, kind="ExternalInput")
    wo_d = nc.dram_tensor("wo_t", [C, C], BF16, kind="ExternalInput")
    bq_d = nc.dram_tensor("bq_c", [128, 8], F32, kind="ExternalInput")
    bk_d = nc.dram_tensor("bk_c", [128, 8], F32, kind="ExternalInput")
    bv_d = nc.dram_tensor("bv_r", [1, C], BF16, kind="ExternalInput")
    bo_d = nc.dram_tensor("bo_r", [1, C], BF16, kind="ExternalInput")
    mb_d = nc.dram_tensor("maskb", [128, 8], F32, kind="ExternalInput")
    on_d = nc.dram_tensor("ones_c", [128, 8, H, 1], BF16, kind="ExternalInput")
    out_d = nc.dram_tensor("out", [T, C], F32, kind="ExternalOutput")
    z_d = nc.dram_tensor("zscratch", [H, T], F32, kind="Internal")
    z2_d = nc.dram_tensor("zscratch2", [H, T], F32, kind="Internal")

    Exp = mybir.ActivationFunctionType.Exp

    with tile.TileContext(nc) as tc:
        with (
            tc.tile_pool(name="singles", bufs=1) as singles,
            tc.tile_pool(name="wpool", bufs=10) as wpool,
            tc.tile_pool(name="acts", bufs=1) as acts,
            tc.tile_pool(name="stream", bufs=3) as stream,
        ):
            # --- small constants ---
            maskb = singles.tile([128, 8], F32)
            nc.gpsimd.dma_start(maskb, mb_d.ap())
            bq_sb = singles.tile([128, 8], F32)
            nc.gpsimd.dma_start(bq_sb, bq_d.ap())
            bk_sb = singles.tile([128, 8], F32)
            nc.gpsimd.dma_start(bk_sb, bk_d.ap())
            if bv_any or bo_any:
                ones1 = singles.tile([1, 128], BF16)
                nc.sync.dma_start(ones1, on_d.ap().rearrange("p a b c -> p (a b c)")[0:1, 0:128])
            if bv_any:
                bv_sb = singles.tile([1, C], BF16)
                nc.sync.dma_start(bv_sb, bv_d.ap())
            if bo_any:
                bo_sb = singles.tile([1, C], BF16)
                nc.sync.dma_start(bo_sb, bo_d.ap())

            # --- persistent activations ---
            # qT_j / later attnT_j share the "qa" slots ([128, 1024] each).
            qT = [
                acts.tile([128, T], BF16, tag="qa", bufs=8, name=f"qT{j}")
                for j in range(8)
            ]
            kT = [
                acts.tile([128, S], BF16, tag="kt", bufs=8, name=f"kT{j}")
                for j in range(8)
            ]
            # v token-major, 65-wide head slots: cols 0..63 = v dims, col 64 = ones
            # (the ones column makes the PV matmul also emit Z = sum_s e as row 64).
            v_sb = acts.tile([128, 8, H, 65], BF16, tag="v", bufs=1)
            ones_col = singles.tile([128, 1], BF16)
            nc.gpsimd.dma_start(
                ones_col, on_d.ap().rearrange("p a b c -> p (a b c)")[:, 0:1]
            )
            nc.vector.tensor_copy(
                v_sb[:, :, :, 64:65], ones_col[:, :, None, None].to_broadcast((128, 8, H, 1))
            )

            # ---------------- Phase A: projections ----------------
            with tc.tile_pool(name="psA", bufs=8, space="PSUM") as psA:

                def proj_featmajor(x_d, w_d, b_sb, outs, wname):
                    # outs[j][o_p, t] = sum_i W.T[i, o] x^T[i, t]  (+ b[o])
                    w_sb = []
                    for k in range(8):
                        wt = wpool.tile([128, C], BF16, tag="w", name=f"{wname}{k}")
                        nc.sync.dma_start(wt, w_d.ap()[k * 128 : (k + 1) * 128, :])
                        w_sb.append(wt)
                    for tci in range(2):
                        ps = [
                            psA.tile([128, 512], F32, tag="pa", name=f"ps{wname}{tci}_{j}")
                            for j in range(8)
                        ]
                        for k in range(8):
                            xch = stream.tile([128, 512], BF16, tag="xch", bufs=4)
                            nc.sync.dma_start(
                                xch,
                                x_d.ap()[
                                    k * 128 : (k + 1) * 128,
                                    tci * 512 : (tci + 1) * 512,
                                ],
                            )
                            for j in range(8):
                                nc.tensor.matmul(
                                    ps[j],
                                    (w_sb[k][:, j * 128 : (j + 1) * 128]),
                                    (xch),
                                    start=(k == 0),
                                    stop=(k == 7),
                                )
                        for j in range(8):
                            nc.vector.tensor_scalar_add(
                                outs[j][:, tci * 512 : (tci + 1) * 512],
                                ps[j],
                                b_sb[:, j : j + 1],
                            )

                proj_featmajor(xq_d, wq_d, bq_sb, qT, "wq")
                proj_featmajor(xk_d, wk_d, bk_sb, kT, "wk")

                # v token-major: v[s, o] = sum_i x^T[i, s] W.T[i, o] (+ bv[o])
                wv_sb = []
                for k in range(8):
                    wt = wpool.tile([128, C], BF16, tag="w", name=f"wv{k}")
                    nc.sync.dma_start(wt, wv_d.ap()[k * 128 : (k + 1) * 128, :])
                    wv_sb.append(wt)
                for sh in range(2):
                    xvh = []
                    for k in range(8):
                        xt = stream.tile(
                            [128, 512], BF16, tag="xvh", bufs=10, name=f"xvh{sh}_{k}"
                        )
                        nc.sync.dma_start(
                            xt,
                            xv_d.ap()[
                                k * 128 : (k + 1) * 128, sh * 512 : (sh + 1) * 512
                            ],
                        )
                        xvh.append(xt)
                    for si in range(4):
                        s = sh * 4 + si
                        for oc in range(2):
                            psv = psA.tile(
                                [128, 512], F32, tag="pa", name=f"psv{s}_{oc}"
                            )
                            for k in range(8):
                                nc.tensor.matmul(
                                    psv,
                                    (xvh[k][:, si * 128 : (si + 1) * 128]),
                                    (wv_sb[k][:, oc * 512 : (oc + 1) * 512]),
                                    start=(k == 0),
                                    stop=(k == 7 and not bv_any),
                                )
                            if bv_any:
                                nc.tensor.matmul(
                                    psv,
                                    (ones1[0:1, 0:128]),
                                    (bv_sb[0:1, oc * 512 : (oc + 1) * 512]),
                                    start=False,
                                    stop=True,
                                )
                            # scatter into per-head 65-wide slots (cols 0..63)
                            nc.vector.tensor_copy(
                                v_sb[:, s, 8 * oc : 8 * oc + 8, 0:64],
                                psv.rearrange("p (h d) -> p h d", d=64),
                            )

            # ---------------- Phase B: attention per head-pair ----------------
            attnT = []
            with tc.tile_pool(name="psB", bufs=2, space="PSUM") as psB:
                for j in range(8):
                    h0, h1 = 2 * j, 2 * j + 1
                    avA = psB.tile([65, T], F32, tag="av", name=f"avA{j}")
                    avB = psB.tile([65, T], F32, tag="av", name=f"avB{j}")
                    for s in range(8):
                        scA = psB.tile([128, T], F32, tag="sc", name=f"scA{j}_{s}")
                        scB = psB.tile([128, T], F32, tag="sc", name=f"scB{j}_{s}")
                        for tcn in range(2):
                            tsl = slice(tcn * 512, (tcn + 1) * 512)
                            nc.tensor.matmul(
                                scA[:, tsl],
                                (kT[j][0:64, s * 128 : (s + 1) * 128]),
                                (qT[j][0:64, tsl]),
                                start=True,
                                stop=True,
                            )
                            nc.tensor.matmul(
                                scB[:, tsl],
                                (kT[j][64:128, s * 128 : (s + 1) * 128]),
                                (qT[j][64:128, tsl]),
                                start=True,
                                stop=True,
                            )
                        eA = stream.tile([128, T], BF16, tag="e", bufs=4, name=f"eA{j}_{s}")
                        eB = stream.tile([128, T], BF16, tag="e", bufs=4, name=f"eB{j}_{s}")
                        nc.scalar.activation(
                            eA, scA, Exp, bias=maskb[:, s : s + 1], scale=SCALE
                        )
                        nc.scalar.activation(
                            eB, scB, Exp, bias=maskb[:, s : s + 1], scale=SCALE
                        )
                        for tcn in range(2):
                            tsl = slice(tcn * 512, (tcn + 1) * 512)
                            nc.tensor.matmul(
                                avA[:, tsl],
                                (v_sb[:, s, h0, :]),
                                (eA[:, tsl]),
                                start=(s == 0),
                                stop=(s == 7),
                            )
                            nc.tensor.matmul(
                                avB[:, tsl],
                                (v_sb[:, s, h1, :]),
                                (eB[:, tsl]),
                                start=(s == 0),
                                stop=(s == 7),
                            )
                    # stash Z rows to DRAM; copy unnormalized num into attnT
                    zrowA = stream.tile([1, T], F32, tag="zz", bufs=3, name=f"zrowA{j}")
                    zrowB = stream.tile([1, T], F32, tag="zz", bufs=3, name=f"zrowB{j}")
                    nc.vector.tensor_copy(zrowA, avA[64:65, :])
                    nc.vector.tensor_copy(zrowB, avB[64:65, :])
                    nc.sync.dma_start(z_d.ap()[h0 : h0 + 1, :], zrowA)
                    nc.sync.dma_start(z_d.ap()[h1 : h1 + 1, :], zrowB)
                    at = acts.tile([128, T], BF16, tag="qa", bufs=8, name=f"attnT{j}")
                    nc.vector.tensor_copy(at[0:64, :], avA[0:64, :])
                    # odd head: copy at base 0, then DMA partition-shift to 64:128
                    tmpB = stream.tile([64, T], BF16, tag="tmpb", bufs=1, name=f"tmpB{j}")
                    nc.vector.tensor_copy(tmpB, avB[0:64, :])
                    nc.sync.dma_start(at[64:128, :], tmpB)
                    attnT.append(at)
                    if j in (3, 7):
                        # batched normalize for pairs lo..j: one fast reciprocal
                        # for 8 heads, then broadcast 1/Z and scale attnT
                        lo = j - 3
                        zall = stream.tile(
                            [8, T], F32, tag="zz", bufs=3, name=f"zall{lo}"
                        )
                        nc.sync.dma_start(zall, z_d.ap()[2 * lo : 2 * lo + 8, :])
                        nc.vector.reciprocal_approx_fast(out=zall, in_=zall)
                        nc.sync.dma_start(z2_d.ap()[2 * lo : 2 * lo + 8, :], zall)
                        for jj in range(lo, j + 1):
                            zbc = stream.tile(
                                [128, T], F32, tag="zz", bufs=3, name=f"zbcn{jj}"
                            )
                            nc.sync.dma_start(
                                zbc[0:64, :],
                                z2_d.ap()[2 * jj : 2 * jj + 1, :].to_broadcast((64, T)),
                            )
                            nc.sync.dma_start(
                                zbc[64:128, :],
                                z2_d.ap()[2 * jj + 1 : 2 * jj + 2, :].to_broadcast(
                                    (64, T)
                                ),
                            )
                            nc.vector.tensor_mul(attnT[jj], attnT[jj], zbc)

            # ---------------- Phase C: output projection ----------------
            wo_sb = []
            for k in range(8):
                wt = wpool.tile([128, C], BF16, tag="w", name=f"wo{k}")
                nc.sync.dma_start(wt, wo_d.ap()[k * 128 : (k + 1) * 128, :])
                wo_sb.append(wt)
            with tc.tile_pool(name="psC", bufs=4, space="PSUM") as psC:
                for tt in range(8):
                    for oc in range(2):
                        pso = psC.tile([128, 512], F32, tag="pc", name=f"pso{tt}_{oc}")
                        for it in range(8):
                            nc.tensor.matmul(
                                pso,
                                (attnT[it][:, tt * 128 : (tt + 1) * 128]),
                                (wo_sb[it][:, oc * 512 : (oc + 1) * 512]),
                                start=(it == 0),
                                stop=(it == 7 and not bo_any),
                            )
                        if bo_any:
                            nc.tensor.matmul(
                                pso,
                                (ones1[0:1, 0:128]),
                                (bo_sb[0:1, oc * 512 : (oc + 1) * 512]),
                                start=False,
                                stop=True,
                            )
                        osb = stream.tile(
                            [128, 512], F32, tag="xch", bufs=4, name=f"osb{tt}_{oc}"
                        )
                        nc.vector.tensor_copy(osb, pso)
                        nc.sync.dma_start(
                            out_d.ap()[
                                tt * 128 : (tt + 1) * 128, oc * 512 : (oc + 1) * 512
                            ],
                            osb,
                        )

    nc.compile()
    return nc


_last_results = None


def kernel(
    query,
    key,
    value,
    key_padding_mask,
    Wq,
    bq,
    Wk,
    bk,
    Wv,
    bv,
    Wo,
    bo,
    _trace=False,
):
    global _last_results
    query = np.asarray(query, np.float32)
    key = np.asarray(key, np.float32)
    value = np.asarray(value, np.float32)
    mask = np.asarray(key_padding_mask, bool)
    Wq = np.asarray(Wq, np.float32)
    Wk = np.asarray(Wk, np.float32)
    Wv = np.asarray(Wv, np.float32)
    Wo = np.asarray(Wo, np.float32)
    bq = np.asarray(bq, np.float32)
    bk = np.asarray(bk, np.float32)
    bv = np.asarray(bv, np.float32)
    bo = np.asarray(bo, np.float32)

    nc = _build(
        bq_any=bool(bq.any()),
        bk_any=bool(bk.any()),
        bv_any=bool(bv.any()),
        bo_any=bool(bo.any()),
    )

    import ml_dtypes

    bf16 = ml_dtypes.bfloat16
    # weight pre-layout (shared across cores): W.T, contiguous [c_in, c_out]
    wqT = np.ascontiguousarray(Wq.T).astype(bf16)
    wkT = np.ascontiguousarray(Wk.T).astype(bf16)
    wvT = np.ascontiguousarray(Wv.T).astype(bf16)
    woT = np.ascontiguousarray(Wo.T).astype(bf16)
    bq_c = np.ascontiguousarray(bq.reshape(8, 128).T)
    bk_c = np.ascontiguousarray(bk.reshape(8, 128).T)
    bv_r = bv.reshape(1, C)
    bo_r = bo.reshape(1, C)

    in_maps = []
    for b in range(N_CORES):
        maskbias = np.where(mask[b], np.float32(-1e30), np.float32(0.0)).astype(
            np.float32
        )
        in_maps.append(
            {
                "xq_t": np.ascontiguousarray(query[:, b, :].T).astype(bf16),
                "xk_t": np.ascontiguousarray(key[:, b, :].T).astype(bf16),
                "xv_t": np.ascontiguousarray(value[:, b, :].T).astype(bf16),
                "wq_t": wqT,
                "wk_t": wkT,
                "wv_t": wvT,
                "wo_t": woT,
                "bq_c": bq_c,
                "bk_c": bk_c,
                "bv_r": bv_r.astype(bf16),
                "bo_r": bo_r.astype(bf16),
                "maskb": np.ascontiguousarray(maskbias.reshape(8, 128).T),
                "ones_c": np.ones((128, 8, H, 1), bf16),
            }
        )

    res = run_bass_kernel_spmd(
        nc,
        in_maps,
        core_ids=list(range(N_CORES)),
        trace=_trace,
    )
    _last_results = res
    out = np.stack([res.results[b]["out"] for b in range(N_CORES)], axis=1)
    return out.astype(np.float32)
